# revision 18
# baseline (speedup 1.0000x reference)
"""GPPT (GCN + prompt MoE routing) Trainium2 kernel, 8-core SPMD. V3.

Row-shards the N=8192 nodes across 8 NeuronCores (1024 rows each).

Per big matmul (adj@X, both GCN layers), instead of 3 fp16 passes:
  corr pass:  Al*Xh + Ah*Xl in ONE fp8 DoubleRow sweep
              (g0=(Xh8, Al8*2^11), g1=(Xl8*2^11, Ah8)) at 0.5 cyc/row,
  main pass:  Ah(fp16) @ Xh(fp16) at 1 cyc/row, with the drained corr
              re-injected via a diag(2^-11) identity matmul (L0) or a DVE
              scaled add (L1).
Effective precision ~1e-5 rel, validated end-to-end in fp64 simulation:
0 routing flips, output rel err ~1e-5 (min top-2 score gap 1.03e-7 abs vs
score std 2.7e-3 -> need ~1.5e-5 rel on the hidden states).

The Y1 AllGather is split into CHUNKS row-chunks, each launched as soon as
its Y1 m-tiles are done; L1's main pass consumes gathered k-tiles chunk by
chunk (A/F rows are permuted on the host into chunk-major order so the
kernel stays rank-agnostic). Gather payload is [yh fp16 | yl8 fp8] = 3 B
per value (the fp8 residual rides the collective bit-cast into fp16 cols).

h0 / Y1 are fp16 3-pass (hi/lo splits, W0 pre-scaled by 2^11 to keep its
lo split in fp16 normal range); expert phase is fp32 as in the baseline.
"""

import os
import numpy as np
import ml_dtypes

import concourse.bass as bass
import concourse.mybir as mybir
import concourse.tile as tile
from concourse import bacc
from concourse.bass_utils import run_bass_kernel_spmd

N = 8192
IN = 512
H = 512
C = 32
E = 7
NCORES = 8
BLK = N // NCORES          # 1024 nodes per core
KT = N // 128              # 64 contraction k-tiles over nodes
SCALE = 8192.0             # adj pre-scale (exact power of two)
W0S = 2048.0               # W0 pre-scale so its fp16 lo split stays normal
CS = 2048.0                # fp8 correction-operand scale (2^11)
CHUNKS = 4                 # AllGather chunks
MTILES = [1, 2, 2, 3]      # Y1 m-tiles per chunk (small first chunk so the
                           # first gather lands before Y1 even finishes)
MOFF = [0, 1, 3, 5]        # running m-tile offset per chunk
KOFF = [0, 8, 24, 40, 64]  # k-tile boundaries per chunk (8*MTILES each)
NW = E + E * C             # 231


def _chunk_of_k(j):
    for c in range(CHUNKS):
        if j < KOFF[c + 1]:
            return c, j - KOFF[c]
    raise AssertionError(j)

F32 = mybir.dt.float32
F16 = mybir.dt.float16
F8 = mybir.dt.float8e4
NP8 = ml_dtypes.float8_e4m3

LAST_RESULTS = None
_CACHED_NC = None


def _kernel_body(ctx, tc, aps):
    nc = tc.nc
    AFT = mybir.ActivationFunctionType
    ALU = mybir.AluOpType
    DR = mybir.MatmulPerfMode.DoubleRow

    A16, A8 = aps["A16"], aps["A8"]
    F16d, F8d = aps["F16"], aps["F8"]
    out = aps["out"]

    const = ctx.enter_context(tc.tile_pool(name="const", bufs=1))
    acts = ctx.enter_context(tc.tile_pool(name="acts", bufs=1))
    ypool = ctx.enter_context(tc.tile_pool(name="ypool", bufs=3))
    small = ctx.enter_context(tc.tile_pool(name="small", bufs=4))
    psum = ctx.enter_context(tc.tile_pool(name="psum", bufs=1, space="PSUM"))

    ps = [psum.tile([128, 512], F32, name=f"bank{i}") for i in range(8)]

    # =================== L0 corr sweep (fp8 DoubleRow) ====================
    with tc.tile_pool(name="l0corr", bufs=12) as cpool:
        pre = {}
        def l0c_load(k):
            a8 = cpool.tile([128, 2, BLK], F8, name="a8")
            f8t = cpool.tile([128, 2, IN], F8, name="f8t")
            r = slice(k * 128, (k + 1) * 128)
            nc.sync.dma_start(a8[:, 0, :], A8[r, 0, :])
            nc.sync.dma_start(a8[:, 1, :], A8[r, 1, :])
            nc.sync.dma_start(f8t[:], F8d[r, :, :])
            return a8, f8t
        for k in range(12):
            pre[k] = l0c_load(k)

        # ---- constants / weights resident in SBUF (ride under the sweep)
        w0h_t, w0l_t, w1h_t, w1l_t = [], [], [], []
        for k in range(4):
            for lst, src in ((w0h_t, "w0h"), (w0l_t, "w0l"),
                             (w1h_t, "w1h"), (w1l_t, "w1l")):
                t = const.tile([128, H], F16, name=f"{src}_{k}")
                nc.sync.dma_start(t[:], aps[src][k * 128:(k + 1) * 128, :])
                lst.append(t)
        wcat_t = []
        for k in range(8):
            t = const.tile([128, NW], F32, name=f"wcat_{k}")
            nc.sync.dma_start(t[:], aps["Wcat"][k * 128:(k + 1) * 128, :])
            wcat_t.append(t)
        b0_t, b1_t = [], []
        for m in range(4):
            t = const.tile([128, 1], F32, name=f"b0_{m}")
            nc.sync.dma_start(t[:], aps["b0"][m * 128:(m + 1) * 128, :])
            b0_t.append(t)
            t = const.tile([128, 1], F32, name=f"b1_{m}")
            nc.sync.dma_start(t[:], aps["b1"][m * 128:(m + 1) * 128, :])
            b1_t.append(t)
        iota_t = const.tile([128, E], F32, name="iota7")
        nc.sync.dma_start(iota_t[:], aps["iota7"][:, :])
        eye_t = const.tile([128, 128], F16, name="eye")
        nc.sync.dma_start(eye_t[:], aps["eye"][:, :])

        for k in range(KT):
            a8, f8t = pre.pop(k) if k in pre else l0c_load(k)
            for m in range(4):
                for n in range(2):
                    nc.tensor.matmul(
                        ps[m * 2 + n][:],
                        f8t[:, :, m * 128:(m + 1) * 128],
                        a8[:, :, n * 512:(n + 1) * 512],
                        start=(k == 0), stop=(k == KT - 1),
                        perf_mode=DR,
                    )

    # corr drain: cs0 = fp16 copy of (corr * CS)
    cs0 = []
    for m in range(4):
        t = acts.tile([128, BLK], F16, name=f"cs0_{m}")
        for n in range(2):
            nc.vector.tensor_copy(t[:, n * 512:(n + 1) * 512], ps[m * 2 + n][:])
        cs0.append(t)

    # =================== L0 main pass (fp16) + inject =====================
    with tc.tile_pool(name="l0main", bufs=4) as mpool:
        for k in range(KT):
            fh = mpool.tile([128, IN], F16, name="fh")
            ah = mpool.tile([128, BLK], F16, name="ah")
            r = slice(k * 128, (k + 1) * 128)
            nc.sync.dma_start(fh[:], F16d[r, :])
            nc.sync.dma_start(ah[:, 0:512], A16[r, 0:512])
            nc.sync.dma_start(ah[:, 512:1024], A16[r, 512:1024])
            for m in range(4):
                for n in range(2):
                    nc.tensor.matmul(
                        ps[m * 2 + n][:],
                        fh[:, m * 128:(m + 1) * 128],
                        ah[:, n * 512:(n + 1) * 512],
                        start=(k == 0), stop=(k == KT - 1),
                    )
            if k == 0:   # inject 2^-11 * cs0 into each bank
                for m in range(4):
                    for n in range(2):
                        nc.tensor.matmul(
                            ps[m * 2 + n][:], eye_t[:],
                            cs0[m][:, n * 512:(n + 1) * 512],
                            start=False, stop=False,
                        )

    # TT = (adj@F)*SCALE now complete in PSUM; drain as fp16 hi/lo split
    tth = [acts.tile([128, BLK], F16, name=f"tth_{m}") for m in range(4)]
    ttl = [acts.tile([128, BLK], F16, name=f"ttl_{m}") for m in range(4)]
    for m in range(4):
        for n in range(2):
            sl = slice(n * 512, (n + 1) * 512)
            nc.vector.tensor_copy(tth[m][:, sl], ps[m * 2 + n][:])
            nc.vector.tensor_tensor(ttl[m][:, sl], ps[m * 2 + n][:],
                                    tth[m][:, sl], op=ALU.subtract)

    # ===== h0 = relu(TT @ W0 / (SCALE*W0S) + b0), fp16 3-pass, interleaved
    # with Y1 = (h0 @ W1)*SCALE per node-half so the gather chunks launch
    # as early as possible. h0 uses even-offset banks, Y1 the others.
    cc_in = [aps[f"cc_in{c}"] for c in range(CHUNKS)]
    cc_out = [aps[f"cc_out{c}"] for c in range(CHUNKS)]
    h0t = [acts.tile([128, BLK], F32, name=f"h0t_{m}") for m in range(4)]
    h0h = [acts.tile([128, BLK], F16, name=f"h0h_{m}") for m in range(4)]
    h0l = [acts.tile([128, BLK], F16, name=f"h0l_{m}") for m in range(4)]
    for half in range(2):
        sl = slice(half * 512, (half + 1) * 512)
        for m in range(4):
            pt = ps[m * 2 + half]
            for p, (wt, tt_) in enumerate(((w0h_t, tth), (w0l_t, tth),
                                           (w0h_t, ttl))):
                for k in range(4):
                    nc.tensor.matmul(
                        pt[:],
                        wt[k][:, m * 128:(m + 1) * 128],
                        tt_[k][:, half * 512:(half + 1) * 512],
                        start=(p == 0 and k == 0),
                        stop=(p == 2 and k == 3),
                    )
            nc.scalar.activation(
                h0t[m][:, sl], pt[:],
                AFT.Relu, bias=b0_t[m][:], scale=1.0 / (SCALE * W0S),
            )
            nc.vector.tensor_copy(h0h[m][:, sl], h0t[m][:, sl])
            nc.vector.tensor_tensor(h0l[m][:, sl], h0t[m][:, sl],
                                    h0h[m][:, sl], op=ALU.subtract)
        for mt in range(half * 4, half * 4 + 4):
            pt = ps[(mt % 4) * 2 + (1 - half)]
            for p, (ht, wt) in enumerate(((h0h, w1h_t), (h0l, w1h_t),
                                          (h0h, w1l_t))):
                for k in range(4):
                    nc.tensor.matmul(
                        pt[:],
                        ht[k][:, mt * 128:(mt + 1) * 128],
                        wt[k][:],
                        start=(p == 0 and k == 0),
                        stop=(p == 2 and k == 3),
                    )
            yh = ypool.tile([128, 512], F16, name="yh")
            ylf = ypool.tile([128, 512], F16, name="ylf")
            yl8 = ypool.tile([128, 512], F8, name="yl8")
            nc.vector.tensor_copy(yh[:], pt[:])
            nc.vector.tensor_tensor(ylf[:], pt[:], yh[:], op=ALU.subtract)
            nc.scalar.activation(yl8[:], ylf[:], AFT.Copy, scale=CS)
            ch = next(c for c in range(CHUNKS) if mt < MOFF[c] + MTILES[c])
            row0 = (mt - MOFF[ch]) * 128
            nc.sync.dma_start(cc_in[ch][row0:row0 + 128, 0:512], yh[:])
            nc.sync.dma_start(cc_in[ch][row0:row0 + 128, 512:768],
                              yl8[:].bitcast(F16))
            if mt == MOFF[ch] + MTILES[ch] - 1:
                nc.gpsimd.collective_compute(
                    "AllGather",
                    mybir.AluOpType.bypass,
                    replica_groups=[list(range(NCORES))],
                    ins=[cc_in[ch].opt()],
                    outs=[cc_out[ch].opt()],
                )

    # ===== expert heads, h_dst (=h0) half — fills the gather-wait hole ====
    oa0 = [acts.tile([128, NW], F32, name=f"oa0_{m}") for m in range(8)]
    for m in range(8):
        pt = ps[m]
        for k in range(4):
            nc.tensor.matmul(
                pt[:, 0:NW],
                h0t[k][:, m * 128:(m + 1) * 128],
                wcat_t[4 + k][:],
                start=(k == 0), stop=(k == 3),
            )
        nc.vector.tensor_copy(oa0[m][:], pt[:, 0:NW])

    # =================== L1 main pass (fp16), chunk-gated =================
    with tc.tile_pool(name="l1main", bufs=4) as mpool:
        for j in range(KT):
            ch, idx = _chunk_of_k(j)
            rr = slice(idx * 128, (idx + 1) * 128)
            yh_t = mpool.tile([128, 512], F16, name="yh_t")
            ah = mpool.tile([128, BLK], F16, name="ah1")
            r = slice(j * 128, (j + 1) * 128)
            nc.sync.dma_start(yh_t[:], cc_out[ch][rr, 0:512])
            nc.sync.dma_start(ah[:, 0:512], A16[r, 0:512])
            nc.sync.dma_start(ah[:, 512:1024], A16[r, 512:1024])
            for m in range(4):
                for n in range(2):
                    nc.tensor.matmul(
                        ps[m * 2 + n][:],
                        yh_t[:, m * 128:(m + 1) * 128],
                        ah[:, n * 512:(n + 1) * 512],
                        start=(j == 0), stop=(j == KT - 1),
                    )

    # drain L1 main to fp32 SBUF
    m_sb = [acts.tile([128, BLK], F32, name=f"msb_{m}") for m in range(4)]
    for m in range(4):
        for n in range(2):
            nc.vector.tensor_copy(m_sb[m][:, n * 512:(n + 1) * 512],
                                  ps[m * 2 + n][:])

    # =================== L1 corr sweep (fp8 DoubleRow) ====================
    with tc.tile_pool(name="l1corr", bufs=6) as cpool:
        for j in range(KT):
            ch, idx = _chunk_of_k(j)
            rr = slice(idx * 128, (idx + 1) * 128)
            r = slice(j * 128, (j + 1) * 128)
            a8 = cpool.tile([128, 2, BLK], F8, name="a8c")
            y8 = cpool.tile([128, 2, 512], F8, name="y8")
            yh_c = cpool.tile([128, 512], F16, name="yh_c")
            nc.sync.dma_start(a8[:, 0, :], A8[r, 0, :])
            nc.sync.dma_start(a8[:, 1, :], A8[r, 1, :])
            nc.sync.dma_start(yh_c[:], cc_out[ch][rr, 0:512])
            nc.sync.dma_start(y8[:, 1, :].bitcast(F16),
                              cc_out[ch][rr, 512:768])
            nc.vector.tensor_copy(y8[:, 0, :], yh_c[:])
            for m in range(4):
                for n in range(2):
                    nc.tensor.matmul(
                        ps[m * 2 + n][:],
                        y8[:, :, m * 128:(m + 1) * 128],
                        a8[:, :, n * 512:(n + 1) * 512],
                        start=(j == 0), stop=(j == KT - 1),
                        perf_mode=DR,
                    )

    # ===== h1 = relu((main + corr/CS)/SCALE^2 + b1) =====
    h1t = [acts.tile([128, BLK], F32, name=f"h1t_{m}") for m in range(4)]
    for m in range(4):
        for n in range(2):
            sl = slice(n * 512, (n + 1) * 512)
            c16 = small.tile([128, 512], F16, name="c16")
            nc.scalar.activation(c16[:], ps[m * 2 + n][:], AFT.Copy,
                                 scale=1.0 / CS)
            hp = small.tile([128, 512], F32, name="hp")
            nc.vector.tensor_tensor(hp[:], m_sb[m][:, sl], c16[:], op=ALU.add)
            nc.scalar.activation(
                h1t[m][:, sl], hp[:],
                AFT.Relu, bias=b1_t[m][:], scale=1.0 / (SCALE * SCALE),
            )

    # ====== expert heads: h (=h1) half + stored h_dst half, then select ===
    for m in range(8):
        pt = ps[m]
        for k in range(4):
            nc.tensor.matmul(
                pt[:, 0:NW],
                h1t[k][:, m * 128:(m + 1) * 128],
                wcat_t[k][:],
                start=(k == 0),
                stop=(k == 3),
            )
        full = small.tile([128, NW], F32, name="full")
        nc.vector.tensor_tensor(full[:], pt[:, 0:NW], oa0[m][:], op=ALU.add)
        sc = full[:, 0:E]
        oa = full[:, E:NW]
        rmax = small.tile([128, 1], F32, name="rmax")
        nc.vector.tensor_reduce(rmax[:], sc, axis=mybir.AxisListType.X,
                                op=ALU.max)
        val = small.tile([128, E], F32, name="val")
        nc.vector.tensor_scalar(val[:], sc, rmax[:], 1024.0, ALU.is_lt,
                                ALU.mult)
        nc.vector.tensor_tensor(val[:], val[:], iota_t[:], op=ALU.add)
        idxf = small.tile([128, 1], F32, name="idxf")
        nc.vector.tensor_reduce(idxf[:], val[:], axis=mybir.AxisListType.X,
                                op=ALU.min)
        onehot = small.tile([128, E], F32, name="onehot")
        nc.vector.tensor_scalar(onehot[:], val[:], idxf[:], None,
                                ALU.is_equal)
        masked = small.tile([128, E, C], F32, name="masked")
        oa_v = oa.rearrange("p (e c) -> p e c", e=E)
        oh_v = onehot[:, :, None].broadcast_to((128, E, C))
        nc.vector.tensor_tensor(masked[:], oa_v, oh_v, op=ALU.mult)
        out_m = small.tile([128, C], F32, name="out_m")
        mv = masked[:].rearrange("p e c -> p c e")
        nc.vector.tensor_reduce(out_m[:], mv, axis=mybir.AxisListType.X,
                                op=ALU.add)
        nc.sync.dma_start(out[m * 128:(m + 1) * 128, :], out_m[:])


def _build_nc():
    nc = bacc.Bacc("TRN2", target_bir_lowering=False, debug=False,
                   num_devices=NCORES)
    aps = {}
    def inp(name, shape, dt):
        aps[name] = nc.dram_tensor(name, shape, dt, kind="ExternalInput").ap()
    inp("A16", [N, BLK], F16)
    inp("A8", [N, 2, BLK], F8)
    inp("F16", [N, IN], F16)
    inp("F8", [N, 2, IN], F8)
    inp("w0h", [IN, H], F16)
    inp("w0l", [IN, H], F16)
    inp("w1h", [H, H], F16)
    inp("w1l", [H, H], F16)
    inp("b0", [H, 1], F32)
    inp("b1", [H, 1], F32)
    inp("Wcat", [2 * H, NW], F32)
    inp("iota7", [128, E], F32)
    inp("eye", [128, 128], F16)
    aps["out"] = nc.dram_tensor("out", [BLK, C], F32,
                                kind="ExternalOutput").ap()
    for c in range(CHUNKS):
        rpc = MTILES[c] * 128
        aps[f"cc_in{c}"] = nc.dram_tensor(f"cc_in{c}", [rpc, 768], F16).ap()
        aps[f"cc_out{c}"] = nc.dram_tensor(
            f"cc_out{c}", [NCORES * rpc, 768], F16, addr_space="Shared").ap()
    from contextlib import ExitStack
    with tile.TileContext(nc) as tc, ExitStack() as ctx:
        _kernel_body(ctx, tc, aps)
    nc.compile()
    return nc


def _split16(x):
    h = x.astype(np.float16)
    l = (x - h.astype(np.float32)).astype(np.float16)
    return h, l


def kernel(feature, adj, W0, b0, W1, b1, Wp, Wpp):
    global LAST_RESULTS, _CACHED_NC
    feature = np.asarray(feature, dtype=np.float32)
    adj = np.asarray(adj, dtype=np.float32)
    W0 = np.asarray(W0, dtype=np.float32)
    b0 = np.asarray(b0, dtype=np.float32)
    W1 = np.asarray(W1, dtype=np.float32)
    b1 = np.asarray(b1, dtype=np.float32)
    Wp = np.asarray(Wp, dtype=np.float32)
    Wpp = np.asarray(Wpp, dtype=np.float32)

    if _CACHED_NC is None:
        _CACHED_NC = _build_nc()
    nc = _CACHED_NC

    # chunk-major row permutation matching the gathered cc_out layout:
    # position j=(ch,idx), rank r=idx//MTILES[ch], sub s=idx%MTILES[ch],
    # global rows r*BLK + (MOFF[ch]+s)*128 + [0,128)
    perm = np.empty(N, dtype=np.int64)
    pos = 0
    for ch in range(CHUNKS):
        for r in range(NCORES):
            for s in range(MTILES[ch]):
                base = r * BLK + (MOFF[ch] + s) * 128
                perm[pos:pos + 128] = np.arange(base, base + 128)
                pos += 128
    assert pos == N

    Fp = feature[perm]
    F16h, F16l = _split16(Fp)
    F8p = np.empty((N, 2, IN), dtype=NP8)
    F8p[:, 0, :] = F16h.astype(NP8)
    F8p[:, 1, :] = (F16l.astype(np.float32) * CS).astype(NP8)

    w0h, w0l = _split16(W0 * W0S)
    w1h, w1l = _split16(W1 * SCALE)
    Wcat = np.concatenate(
        [Wp, Wpp.transpose(1, 0, 2).reshape(2 * H, E * C)], axis=1)
    iota7 = np.tile(np.arange(E, dtype=np.float32), (128, 1))
    eye = (np.eye(128, dtype=np.float32) / CS).astype(np.float16)
    shared = {
        "F16": F16h, "F8": F8p,
        "w0h": w0h, "w0l": w0l, "w1h": w1h, "w1l": w1l,
        "b0": b0.reshape(H, 1), "b1": b1.reshape(H, 1),
        "Wcat": np.ascontiguousarray(Wcat), "iota7": iota7, "eye": eye,
    }
    in_maps = []
    for c in range(NCORES):
        blk = adj[c * BLK:(c + 1) * BLK, :].T.astype(np.float32) * SCALE
        blk = blk[perm, :]
        A_h, A_l = _split16(blk)
        A8p = np.empty((N, 2, BLK), dtype=NP8)
        A8p[:, 0, :] = (A_l.astype(np.float32) * CS).astype(NP8)
        A8p[:, 1, :] = A_h.astype(NP8)
        m = dict(shared)
        m["A16"] = np.ascontiguousarray(A_h)
        m["A8"] = A8p
        in_maps.append(m)

    trace = os.environ.get("BASS_KERNEL_TRACE", "0") == "1"
    res = run_bass_kernel_spmd(nc, in_maps, list(range(NCORES)), trace=trace)
    LAST_RESULTS = res
    out = np.concatenate([res.results[c]["out"] for c in range(NCORES)],
                         axis=0)
    return out
